# revision 1
# baseline (speedup 1.0000x reference)
"""Trainium2 Bass kernel for nn_CORALLoss (RAL + OAL loss over n=512 samples).

Strategy: shard the anchor dimension (512 rows) across 8 cores — each core
handles 32 anchors from view-0 and their 32 view-1 partners (partners share
identical |p_i - p_j| rows, so each comparison mask serves two anchors).
Per anchor pair the vector engine builds the [k, j] comparison mask
  mask[k, j] = (pd[i, j] <= pd[i, k])
in 4 chunks of [128, 512], and the tensor engine contracts each chunk with
the pair's exp-similarity columns into a per-pair PSUM [2, 512]
denominator, which the scalar engine logs into a [2, 32, 512] staging tile
(one DMA later rearranges it row-major). Everything else (normalization,
OAL pairwise distances) is matmuls plus a handful of DVE/ACT ops.

The instruction mix is arranged so every compute instruction needs at most
ONE semaphore wait (the TPB instruction encoding has a single wait slot):
each engine's first instruction touching the input blob carries the DMA
wait, per-pair "observer" matmuls absorb the ACT slot-release wait before
the accumulation chain, and cross-engine consumers are ordered so the
second operand always comes from the same engine or an already-observed
semaphore. Per-core partial sums are combined on the host.
"""
import sys
from contextlib import ExitStack

import numpy as np

sys.path.insert(0, "/opt/trn_rl_repo")

import concourse.bass as bass
import concourse.mybir as mybir
from concourse import tile
from concourse.bass_utils import run_bass_kernel_spmd

AF = mybir.ActivationFunctionType
OP = mybir.AluOpType
F32 = mybir.dt.float32

N, D, NCORES, HALF = 512, 128, 8, 32
PBUF = 3
TEMP = 0.07
EPS = 1e-8

# blob column layout (single packed [128, BLOB] f32 input)
_c = 0
def _span(w):
    global _c
    s = (_c, _c + w)
    _c += w
    return s

C_CFT = _span(512)     # cfT full features, d-major
C_CFTR = _span(64)     # this core's anchor columns
C_PCOL = _span(4)      # labels chunk-column layout
C_PROWB = _span(64)    # anchor labels bcast on all partitions
C_V = _span(1)         # v_prog column
C_VREP = _span(64)     # v_prog replicated
C_ONES = _span(128)    # ones block
C_TDIAG = _span(256)   # diag complement for eT chunks
C_EPS = _span(1)       # EPS column
C_PROW512 = _span(512) # row 0 = tiled labels
C_PROWC = _span(1)     # partitions 0:64 = anchor labels
C_VDIAG = _span(512)   # partitions 0:64 = diag complement row-major
BLOB = _c

_CACHE = {}


def _build_program(n_reps=1):
    nc = bass.Bass()
    blob_d = nc.declare_dram_parameter("blob", [128, BLOB], F32, isOutput=False)
    out_d = nc.declare_dram_parameter("partials", [1, 8], F32, isOutput=True)

    with tile.TileContext(nc) as tc, ExitStack() as ctx:
        const = ctx.enter_context(tc.tile_pool(name="const", bufs=1))
        work = ctx.enter_context(tc.tile_pool(name="work", bufs=1))
        maskp = ctx.enter_context(tc.tile_pool(name="maskp", bufs=6))
        pdbp = ctx.enter_context(tc.tile_pool(name="pdbp", bufs=2))
        psB = ctx.enter_context(tc.tile_pool(name="psB", bufs=2, space="PSUM"))
        psS = ctx.enter_context(tc.tile_pool(name="psS", bufs=2, space="PSUM"))
        psP = ctx.enter_context(tc.tile_pool(name="psP", bufs=PBUF, space="PSUM"))
        psO = ctx.enter_context(tc.tile_pool(name="psO", bufs=1, space="PSUM"))

        blob = const.tile([128, BLOB], F32, tag="blob")
        nc.gpsimd.dma_start(blob[:], blob_d[:])

        # First PE / ACT instructions touch only the blob, so they carry the
        # single DMA wait and later instructions inherit the observed tick.
        for _rep in range(n_reps):
            _emit_body(nc, const, work, maskp, pdbp, psB, psS, psP, psO,
                       blob, out_d, _rep, _rep == n_reps - 1)

    _split_multiwaits(nc)
    return nc


def _emit_body(nc, const, work, maskp, pdbp, psB, psS, psP, psO, blob, out_d, rep=0, last=True):
        def bs(span, p0=0, p1=128):
            return blob[p0:p1, span[0]:span[1]]

        cfT = bs(C_CFT)
        cfTr = bs(C_CFTR)
        p_col = bs(C_PCOL)
        prowb = bs(C_PROWB)
        v = bs(C_V)
        vrep = bs(C_VREP)
        ones1 = bs(C_ONES, 0, 1)
        onesr = blob[:, C_ONES[0]:C_ONES[0] + 64]
        ones128 = blob[:, C_ONES[0]:C_ONES[0] + 1]
        ones64 = blob[0:64, C_ONES[0]:C_ONES[0] + 1]
        tdiag = bs(C_TDIAG)
        epsc = bs(C_EPS)
        p_row = bs(C_PROW512, 0, 1)
        prow = bs(C_PROWC, 0, 64)
        vdiag = bs(C_VDIAG, 0, 64)

        vsq_ps = psS.tile([1, 1], F32, tag="small")
        nc.tensor.matmul(vsq_ps[:], v, v, start=True, stop=True)
        eps_sb = const.tile([2, 1], F32, tag="eps_sb")
        nc.scalar.copy(eps_sb[:], epsc[0:2, 0:1])
        out_tile = const.tile([1, 8], F32, tag="out_tile")
        vsq_sb = work.tile([1, 1], F32, tag="vsq_sb")
        nc.vector.tensor_copy(vsq_sb[:], vsq_ps[:])
        lnv = work.tile([1, 1], F32, tag="lnv")
        nc.scalar.activation(lnv[:], vsq_sb[:], AF.Ln)
        nc.scalar.activation(out_tile[0:1, 3:4], lnv[:], AF.Exp, scale=-0.5)

        # ---------------- normalization ----------------
        sq = work.tile([128, 512], F32, tag="sq")
        nc.vector.tensor_tensor(sq[:], cfT, cfT, op=OP.mult)
        sq_r = work.tile([128, 64], F32, tag="sq_r")
        nc.vector.tensor_tensor(sq_r[:], cfTr, cfTr, op=OP.mult)

        sqnb_ps = psB.tile([64, 512], F32, tag="big")   # sqn_j bcast over rows
        nc.tensor.matmul(sqnb_ps[:], onesr, sq[:], start=True, stop=True)
        sqnr512_ps = psS.tile([1, 512], F32, tag="small")
        nc.tensor.matmul(sqnr512_ps[:], ones128, sq[:], start=True, stop=True)
        sqn512_sb = work.tile([1, 512], F32, tag="sqn512_sb")
        nc.vector.tensor_copy(sqn512_sb[:], sqnr512_ps[:])
        lnn = work.tile([1, 512], F32, tag="lnn")
        nc.scalar.activation(lnn[:], sqn512_sb[:], AF.Ln)
        invn = work.tile([1, 512], F32, tag="invn")
        nc.scalar.activation(invn[:], lnn[:], AF.Exp, scale=-0.5)
        sqnb_sb = const.tile([64, 512], F32, tag="sqnb_sb")
        nc.vector.tensor_copy(sqnb_sb[:], sqnb_ps[:])

        sqnr_ps = psS.tile([1, 64], F32, tag="small")   # row layout for invn_r
        nc.tensor.matmul(sqnr_ps[:], ones128, sq_r[:], start=True, stop=True)
        sqnr64_sb = work.tile([1, 64], F32, tag="sqnr64_sb")
        nc.vector.tensor_copy(sqnr64_sb[:], sqnr_ps[:])
        lnr = work.tile([1, 64], F32, tag="lnr")
        nc.scalar.activation(lnr[:], sqnr64_sb[:], AF.Ln)
        invn_r = work.tile([1, 64], F32, tag="invn_r")
        nc.scalar.activation(invn_r[:], lnr[:], AF.Exp, scale=-0.5)

        sqnc_ps = psS.tile([64, 1], F32, tag="small")   # sqn of anchor rows
        nc.tensor.matmul(sqnc_ps[:], sq_r[:], ones128, start=True, stop=True)
        sqnc_sb = const.tile([64, 1], F32, tag="sqnc_sb")
        nc.vector.tensor_copy(sqnc_sb[:], sqnc_ps[:])

        invnb_ps = psB.tile([128, 512], F32, tag="big")
        nc.tensor.matmul(invnb_ps[:], ones1, invn[:], start=True, stop=True)
        fT = const.tile([128, 512], F32, tag="fT")
        nc.vector.tensor_tensor(fT[:], cfT, invnb_ps[:], op=OP.mult)
        invnrb_ps = psB.tile([128, 64], F32, tag="big")
        nc.tensor.matmul(invnrb_ps[:], ones1, invn_r[:], start=True, stop=True)
        fTr = const.tile([128, 64], F32, tag="fTr")
        nc.vector.tensor_tensor(fTr[:], cfTr, invnrb_ps[:], op=OP.mult)

        # p broadcast [128, 512] (p_j along free dim on every partition)
        pb_ps = psB.tile([128, 512], F32, tag="big")
        nc.tensor.matmul(pb_ps[:], ones1, p_row, start=True, stop=True)
        p_b = const.tile([128, 512], F32, tag="p_b")
        nc.scalar.copy(p_b[:], pb_ps[:])

        negprowb = const.tile([128, 64], F32, tag="negprowb")
        nc.vector.tensor_scalar(negprowb[:], prowb, -1.0, None, op0=OP.mult)
        negp_col = const.tile([128, 4], F32, tag="negp_col")
        nc.vector.tensor_scalar(negp_col[:], p_col, -1.0, None, op0=OP.mult)
        negprow = const.tile([64, 1], F32, tag="negprow")
        nc.vector.tensor_scalar(negprow[:], prow, -1.0, None, op0=OP.mult)

        # ---------------- exp-similarity columns (transposed) ----------------
        eT = []
        for q in range(4):
            gT_ps = psB.tile([128, 64], F32, tag="big")
            nc.tensor.matmul(gT_ps[:], fT[:, 128 * q:128 * (q + 1)], fTr[:],
                             start=True, stop=True)
            e_q = const.tile([128, 64], F32, tag=f"eT{q}")
            nc.scalar.activation(e_q[:], gT_ps[:], AF.Exp, scale=1.0 / TEMP)
            nc.vector.tensor_tensor(e_q[:], e_q[:],
                                    tdiag[:, 64 * q:64 * (q + 1)], op=OP.mult)
            eT.append(e_q)

        # ---------------- pd structures ----------------
        pdT = []
        for q in range(4):
            t = const.tile([128, 64], F32, tag=f"pdT{q}")
            nc.scalar.activation(t[:], prowb, AF.Abs,
                                 bias=negp_col[:, q:q + 1])
            pdT.append(t)

        pd_rows = const.tile([64, 512], F32, tag="pd_rows")
        nc.scalar.activation(pd_rows[:], p_b[0:64, :], AF.Abs,
                             bias=negprow[:])

        th = work.tile([64, 512], F32, tag="th")
        nc.scalar.activation(th[:], pd_rows[:], AF.Tanh, scale=0.5)
        sw = work.tile([64, 512], F32, tag="sw")
        nc.vector.tensor_scalar(sw[:], th[:], 0.5, 0.5, op0=OP.mult, op1=OP.add)
        soft_wm = const.tile([64, 512], F32, tag="soft_wm")
        nc.vector.tensor_tensor(soft_wm[:], sw[:], vdiag, op=OP.mult)

        pmask = const.tile([64, 512], F32, tag="pmask")
        cnt_rows = const.tile([64, 1], F32, tag="cnt_rows")
        nc.vector.tensor_scalar(pmask[:], p_b[0:64, :], prow, None,
                                op0=OP.is_gt, op1=OP.add, accum_out=cnt_rows[:])

        # ---------------- OAL ----------------
        projc_ps = psS.tile([64, 1], F32, tag="small")
        nc.tensor.matmul(projc_ps[:], cfTr, v, start=True, stop=True)
        projc_sb = const.tile([64, 1], F32, tag="projc_sb")
        nc.vector.tensor_copy(projc_sb[:], projc_ps[:])

        projb_ps = psB.tile([64, 512], F32, tag="big")
        nc.tensor.matmul(projb_ps[:], vrep, cfT, start=True, stop=True)
        t3a = work.tile([64, 512], F32, tag="t3a")
        nc.vector.scalar_tensor_tensor(t3a[:], projb_ps[:], projc_sb[:],
                                       pmask[:], op0=OP.subtract, op1=OP.mult)

        rgram_ps = psB.tile([64, 512], F32, tag="big")
        nc.tensor.matmul(rgram_ps[:], cfTr, cfT, start=True, stop=True)
        t1 = work.tile([64, 512], F32, tag="t1")
        nc.vector.scalar_tensor_tensor(t1[:], rgram_ps[:], -2.0, sqnb_sb[:],
                                       op0=OP.mult, op1=OP.add)
        sqd0 = work.tile([64, 512], F32, tag="sqd0")
        nc.vector.tensor_scalar(sqd0[:], t1[:], sqnc_sb[:], None, op0=OP.add)
        sqd = work.tile([64, 512], F32, tag="sqd")
        nc.vector.tensor_scalar(sqd[:], sqd0[:], 1e-24, None, op0=OP.max)
        lnd = work.tile([64, 512], F32, tag="lnd")
        nc.scalar.activation(lnd[:], sqd[:], AF.Ln)
        invd = work.tile([64, 512], F32, tag="invd")
        nc.scalar.activation(invd[:], lnd[:], AF.Exp, scale=-0.5)

        t4 = work.tile([64, 512], F32, tag="t4")
        oal_rows = const.tile([64, 1], F32, tag="oal_rows")
        nc.vector.scalar_tensor_tensor(t4[:], t3a[:], 1.0, invd[:],
                                       op0=OP.mult, op1=OP.mult,
                                       accum_out=oal_rows[:])

        oal_ps = psS.tile([1, 1], F32, tag="small")
        nc.tensor.matmul(oal_ps[:], oal_rows[:], ones64, start=True, stop=True)
        nc.scalar.copy(out_tile[0:1, 1:2], oal_ps[:])
        cnt_ps = psS.tile([1, 1], F32, tag="small")
        nc.tensor.matmul(cnt_ps[:], cnt_rows[:], ones64, start=True, stop=True)
        nc.scalar.copy(out_tile[0:1, 2:3], cnt_ps[:])

        # gram of normalized rows, evacuated to SBUF before the pair loop
        gram_ps = psB.tile([64, 512], F32, tag="big")
        nc.tensor.matmul(gram_ps[:], fTr[:], fT[:], start=True, stop=True)
        gram_sb = const.tile([64, 512], F32, tag="gram_sb")
        nc.vector.tensor_copy(gram_sb[:], gram_ps[:])

        # ---------------- denominator pair loop ----------------
        obs_ps = psO.tile([1, 1], F32, tag="obs")
        dveobs = const.tile([1, 2], F32, tag="dveobs")
        ld_all = const.tile([2, HALF, 512], F32, tag=f"ld_all{rep % 2}")
        for r in range(HALF):
            pd_b = pdbp.tile([128, 512], F32, tag="pd_b")
            nc.scalar.activation(pd_b[:], p_b[:], AF.Abs,
                                 bias=negprowb[:, r:r + 1])
            # DVE observer: absorbs the ACT(pd_b) wait so the mask ops
            # below need only their PE slot-release wait.
            nc.vector.tensor_copy(dveobs[:], pd_b[0:1, 0:2])
            if r >= PBUF:
                # PE observer: waits on the Ln that frees this pair's PSUM
                # slot, so the q==0 matmul below needs only its DVE wait.
                nc.tensor.matmul(obs_ps[:], ld_all[0:2, r - PBUF, 0:1],
                                 ld_all[0:2, r - PBUF, 0:1],
                                 start=True, stop=True)
            pr_ps = psP.tile([2, 512], F32, tag="pair")
            for q in range(4):
                mk = maskp.tile([128, 512], F32, tag="mk")
                nc.vector.tensor_scalar(mk[:], pd_b[:], pdT[q][:, r:r + 1],
                                        None, op0=OP.is_le)
                nc.tensor.matmul(pr_ps[:], eT[q][:, r:r + HALF + 1:HALF], mk[:],
                                 start=(q == 0), stop=(q == 3))
            nc.scalar.activation(ld_all[0:2, r, :], pr_ps[:],
                                 AF.Ln, bias=eps_sb[:])
        ldenom = const.tile([64, 512], F32, tag=f"ldenom{rep % 2}")
        nc.sync.dma_start(ldenom[:], ld_all[:])
        # ACT observer of the assembly DMA so next-rep ACT writes to ld_all
        # need no extra DMA wait
        ldobs = const.tile([1, 2], F32, tag="ldobs")
        nc.scalar.copy(ldobs[:], ldenom[0:1, 0:2])

        # ---------------- RAL finalize ----------------
        t5 = work.tile([64, 512], F32, tag="t5")
        nc.vector.scalar_tensor_tensor(t5[:], gram_sb[:], -1.0 / TEMP, ldenom[:],
                                       op0=OP.mult, op1=OP.add)
        contrib = work.tile([64, 512], F32, tag="contrib")
        ral_rows = const.tile([64, 1], F32, tag="ral_rows")
        nc.vector.scalar_tensor_tensor(contrib[:], t5[:], 1.0, soft_wm[:],
                                       op0=OP.mult, op1=OP.mult,
                                       accum_out=ral_rows[:])
        ral_ps = psB.tile([1, 1], F32, tag="big")
        nc.tensor.matmul(ral_ps[:], ral_rows[:], ones64, start=True, stop=True)
        nc.scalar.copy(out_tile[0:1, 0:1], ral_ps[:])

        if last:
            nc.sync.dma_start(out_d[0:1, 0:4], out_tile[0:1, 0:4])


def _split_multiwaits(nc):
    """TPB instructions encode a single semaphore wait, but Tile emits up
    to one wait per dependency. Two legalizations: (1) drop same-engine
    self-waits that are provably satisfied (the engine completes its own
    instructions in order, so a wait on its own semaphore for a value
    already accumulated upstream is vacuous; DMA-queue semaphores are
    excluded since their increments fire on async transfer completion);
    (2) the kernel-tail drain keeps only the final output-DMA completion
    wait — every other wait is transitively implied by the PE -> ACT ->
    out-DMA chain."""
    eng_sem = {
        "EngineType.DVE": "DVE",
        "EngineType.Activation": "Activation",
        "EngineType.PE": "PE",
        "EngineType.Pool": "Pool",
        "EngineType.SP": "SP",
    }
    fn = nc.m.functions[0]
    streams = {}
    for blk in fn.blocks:
        for ins in blk.instructions:
            streams.setdefault(str(getattr(ins, "engine", None)), []).append(ins)
    for eng, insts in streams.items():
        own = eng_sem.get(eng)
        cum = {}
        last_dma_updates = set()
        for ins in insts:
            si = ins.sync_info
            if si is None:
                continue
            if type(ins).__name__ == "InstDMACopy":
                last_dma_updates = {u.id for u in si.on_update}
                own_q = {u.id for u in si.on_update}
                if len(si.on_wait) > 2:
                    # same-queue waits on earlier transfers are vacuous
                    # (HWDGE queues execute and complete in order)
                    new = [w for w in si.on_wait
                           if not (w.id in own_q
                                   and w.wait_value <= cum.get(w.id, 0))]
                    if len(new) > 1:
                        # ldenom assembly DMA across reps: its DVE wait
                        # (t5 of rep N-2 reading ldenom) and prior-rep DMA
                        # WAW wait are both transitively implied by its ACT
                        # wait: ld_all writes of rep N come after rep N-2's
                        # ral copy -> PE ral matmul -> contrib -> t5, and
                        # t5 itself waited on rep N-2's assembly DMA.
                        names = [w.ant_name for w in new]
                        assert any(n.startswith("Activation") for n in names), names
                        new = [w for w in new
                               if w.ant_name.startswith("Activation")]
                    assert len(new) <= 1, (
                        f"{ins.name} DMA still needs "
                        f"{[w.ant_name for w in new]}")
                    si.on_wait = new
                    ins.sync_info = si
                for u in si.on_update:
                    cum[u.id] = cum.get(u.id, 0) + u.update_value
                continue
            if len(si.on_wait) > 1:
                if type(ins).__name__ == "InstDrain":
                    kept = [w for w in si.on_wait if w.id in last_dma_updates]
                    assert kept, f"no DMA-completion wait for {ins.name}"
                    si.on_wait = kept[-1:]
                    ins.sync_info = si
                else:
                    new = [
                        w for w in si.on_wait
                        if not (own and w.ant_name.startswith(own + "_")
                                and w.wait_value <= cum.get(w.id, 0))
                    ]
                    assert len(new) <= 1, (
                        f"{ins.name} on {eng} still needs "
                        f"{[w.ant_name for w in new]}"
                    )
                    si.on_wait = new
                    ins.sync_info = si
            for u in si.on_update:
                cum[u.id] = cum.get(u.id, 0) + u.update_value


def _host_inputs(features, labels, v_prog):
    f32 = np.float32
    cf = np.ascontiguousarray(
        features.astype(f32).transpose(1, 0, 2).reshape(N, D))
    p = np.concatenate([labels, labels]).astype(f32)
    cfT = np.ascontiguousarray(cf.T)
    v = v_prog.astype(f32).reshape(128, 1)

    base = np.zeros((128, BLOB), f32)
    base[:, C_CFT[0]:C_CFT[1]] = cfT
    base[:, C_PCOL[0]:C_PCOL[1]] = p.reshape(4, 128).T
    base[:, C_V[0]:C_V[1]] = v
    base[:, C_VREP[0]:C_VREP[1]] = v
    base[:, C_ONES[0]:C_ONES[1]] = 1.0
    base[:, C_EPS[0]:C_EPS[1]] = EPS
    base[0, C_PROW512[0]:C_PROW512[1]] = p

    in_maps = []
    for c in range(NCORES):
        rows = np.r_[32 * c:32 * c + 32, 256 + 32 * c:256 + 32 * c + 32]
        prow = p[rows]
        blob = base.copy()
        blob[:, C_CFTR[0]:C_CFTR[1]] = cf[rows].T
        blob[:, C_PROWB[0]:C_PROWB[1]] = prow[None, :]
        blob[:64, C_PROWC[0]] = prow
        tdiag = np.ones((128, 256), f32)
        vdiag = np.ones((64, N), f32)
        for r, g in enumerate(rows):
            q, kp = divmod(g, 128)
            tdiag[kp, 64 * q + r] = 0.0
            vdiag[r, g] = 0.0
        blob[:, C_TDIAG[0]:C_TDIAG[1]] = tdiag
        blob[:64, C_VDIAG[0]:C_VDIAG[1]] = vdiag
        in_maps.append({"blob": blob})
    return in_maps


def _get_program(n_reps=1):
    key = ("nc", n_reps)
    if key not in _CACHE:
        _CACHE[key] = _build_program(n_reps)
    return _CACHE[key]


def kernel(features, labels, v_prog, _bench=None, _n_reps=1):
    nc = _get_program(_n_reps)
    in_maps = _host_inputs(np.asarray(features), np.asarray(labels),
                           np.asarray(v_prog))
    kw = dict(_bench or {})
    res = run_bass_kernel_spmd(nc, in_maps, list(range(NCORES)), **kw)
    parts = np.stack([res.results[c]["partials"][0] for c in range(NCORES)])
    f32 = np.float32
    ral = parts[:, 0].sum(dtype=f32) / f32(N * (N - 1))
    oal_num = (parts[:, 1] * parts[:, 3]).sum(dtype=f32)
    oal = -oal_num / parts[:, 2].sum(dtype=f32)
    out = np.float32(ral + oal)
    if _bench is not None:
        return out, res
    return out



# revision 6
# speedup vs baseline: 73.6591x; 73.6591x over previous
"""Trainium2 Bass kernel for nn_CORALLoss (RAL + OAL loss over n=512 samples).

Strategy: shard the anchor dimension (512 rows) across 8 cores — each core
handles 32 anchors from view-0 and their 32 view-1 partners (partners share
identical |p_i - p_j| rows, so each comparison mask serves two anchors).
Per anchor pair the vector engine builds the [k, j] comparison mask
  mask[k, j] = (pd[i, j] <= pd[i, k])
in 4 chunks of [128, 512], and the tensor engine contracts each chunk with
the pair's exp-similarity columns into a per-pair PSUM [2, 512]
denominator, which the scalar engine logs into a [2, 32, 512] staging tile
(one DMA later rearranges it row-major). Everything else (normalization,
OAL pairwise distances) is matmuls plus a handful of DVE/ACT ops.

The instruction mix is arranged so every compute instruction needs at most
ONE semaphore wait (the TPB instruction encoding has a single wait slot):
each engine's first instruction touching the input blob carries the DMA
wait, per-pair "observer" matmuls absorb the ACT slot-release wait before
the accumulation chain, and cross-engine consumers are ordered so the
second operand always comes from the same engine or an already-observed
semaphore. Per-core partial sums are combined on the host.
"""
import sys
from contextlib import ExitStack

import numpy as np

sys.path.insert(0, "/opt/trn_rl_repo")

import concourse.bass as bass
import concourse.bacc as bacc
import concourse.mybir as mybir
from concourse import tile
from concourse.bass_utils import run_bass_kernel_spmd

AF = mybir.ActivationFunctionType
OP = mybir.AluOpType
F32 = mybir.dt.float32

N, D, NCORES, HALF = 512, 128, 8, 32
PBUF = 3
TEMP = 0.07
EPS = 1e-8

# blob column layout (single packed [128, BLOB] f32 input)
_c = 0
def _span(w):
    global _c
    s = (_c, _c + w)
    _c += w
    return s

C_CFT = _span(512)     # cfT full features, d-major
C_CFTR = _span(64)     # this core's anchor columns
C_PCOL = _span(4)      # labels chunk-column layout
C_PROWB = _span(64)    # anchor labels bcast on all partitions
C_V = _span(1)         # v_prog column
C_VREP = _span(64)     # v_prog replicated
C_ONES = _span(128)    # ones block
C_TDIAG = _span(256)   # diag complement for eT chunks
C_EPS = _span(1)       # EPS column
C_PROW512 = _span(512) # row 0 = tiled labels
C_PROWC = _span(1)     # partitions 0:64 = anchor labels
C_VDIAG = _span(512)   # partitions 0:64 = diag complement row-major
BLOB = _c

_CACHE = {}


def _build_program(n_reps=1):
    # Bacc: Bass + compiler passes (finalize() splits multi-waits into the
    # single wait slot TPB instructions encode, handles control flow).
    nc = bacc.Bacc()
    blob_d = nc.declare_dram_parameter("blob", [128, BLOB], F32, isOutput=False)
    out_d = nc.declare_dram_parameter("partials", [1, 8], F32, isOutput=True)

    with tile.TileContext(nc) as tc, ExitStack() as ctx:
        const = ctx.enter_context(tc.tile_pool(name="const", bufs=1))
        work = ctx.enter_context(tc.tile_pool(name="work", bufs=1))
        maskp = ctx.enter_context(tc.tile_pool(name="maskp", bufs=6))
        pdbp = ctx.enter_context(tc.tile_pool(name="pdbp", bufs=2))
        psB = ctx.enter_context(tc.tile_pool(name="psB", bufs=2, space="PSUM"))
        psS = ctx.enter_context(tc.tile_pool(name="psS", bufs=2, space="PSUM"))
        psP = ctx.enter_context(tc.tile_pool(name="psP", bufs=PBUF, space="PSUM"))
        psO = ctx.enter_context(tc.tile_pool(name="psO", bufs=1, space="PSUM"))

        blob = const.tile([128, BLOB], F32, tag="blob")
        nc.gpsimd.dma_start(blob[:], blob_d[:])

        # Hardware loop: one copy of the body regardless of n_reps, so the
        # program (and its compile/load cost) is rep-count independent and
        # wall-clock slope over n_reps measures pure device execution.
        if n_reps == 1:
            _emit_body(nc, const, work, maskp, pdbp, psB, psS, psP, psO,
                       blob, out_d, 0, True)
        else:
            with tc.For_i(0, n_reps, 1):
                _emit_body(nc, const, work, maskp, pdbp, psB, psS, psP, psO,
                           blob, out_d, 0, True)

    nc.finalize()
    return nc


def _emit_body(nc, const, work, maskp, pdbp, psB, psS, psP, psO, blob, out_d, rep=0, last=True):
        def bs(span, p0=0, p1=128):
            return blob[p0:p1, span[0]:span[1]]

        cfT = bs(C_CFT)
        cfTr = bs(C_CFTR)
        p_col = bs(C_PCOL)
        prowb = bs(C_PROWB)
        v = bs(C_V)
        vrep = bs(C_VREP)
        ones1 = bs(C_ONES, 0, 1)
        onesr = blob[:, C_ONES[0]:C_ONES[0] + 64]
        ones128 = blob[:, C_ONES[0]:C_ONES[0] + 1]
        ones64 = blob[0:64, C_ONES[0]:C_ONES[0] + 1]
        tdiag = bs(C_TDIAG)
        epsc = bs(C_EPS)
        p_row = bs(C_PROW512, 0, 1)
        prow = bs(C_PROWC, 0, 64)
        vdiag = bs(C_VDIAG, 0, 64)

        vsq_ps = psS.tile([1, 1], F32, tag="small")
        nc.tensor.matmul(vsq_ps[:], v, v, start=True, stop=True)
        eps_sb = const.tile([2, 1], F32, tag="eps_sb")
        nc.scalar.copy(eps_sb[:], epsc[0:2, 0:1])
        out_tile = const.tile([1, 8], F32, tag="out_tile")
        vsq_sb = work.tile([1, 1], F32, tag="vsq_sb")
        nc.vector.tensor_copy(vsq_sb[:], vsq_ps[:])
        lnv = work.tile([1, 1], F32, tag="lnv")
        nc.scalar.activation(lnv[:], vsq_sb[:], AF.Ln)
        nc.scalar.activation(out_tile[0:1, 3:4], lnv[:], AF.Exp, scale=-0.5)

        # ---------------- normalization ----------------
        sq = work.tile([128, 512], F32, tag="sq")
        nc.vector.tensor_tensor(sq[:], cfT, cfT, op=OP.mult)
        sq_r = work.tile([128, 64], F32, tag="sq_r")
        nc.vector.tensor_tensor(sq_r[:], cfTr, cfTr, op=OP.mult)

        sqnb_ps = psB.tile([64, 512], F32, tag="big")   # sqn_j bcast over rows
        nc.tensor.matmul(sqnb_ps[:], onesr, sq[:], start=True, stop=True)
        sqnr512_ps = psS.tile([1, 512], F32, tag="small")
        nc.tensor.matmul(sqnr512_ps[:], ones128, sq[:], start=True, stop=True)
        sqn512_sb = work.tile([1, 512], F32, tag="sqn512_sb")
        nc.vector.tensor_copy(sqn512_sb[:], sqnr512_ps[:])
        lnn = work.tile([1, 512], F32, tag="lnn")
        nc.scalar.activation(lnn[:], sqn512_sb[:], AF.Ln)
        invn = work.tile([1, 512], F32, tag="invn")
        nc.scalar.activation(invn[:], lnn[:], AF.Exp, scale=-0.5)
        sqnb_sb = const.tile([64, 512], F32, tag="sqnb_sb")
        nc.vector.tensor_copy(sqnb_sb[:], sqnb_ps[:])

        sqnr_ps = psS.tile([1, 64], F32, tag="small")   # row layout for invn_r
        nc.tensor.matmul(sqnr_ps[:], ones128, sq_r[:], start=True, stop=True)
        sqnr64_sb = work.tile([1, 64], F32, tag="sqnr64_sb")
        nc.vector.tensor_copy(sqnr64_sb[:], sqnr_ps[:])
        lnr = work.tile([1, 64], F32, tag="lnr")
        nc.scalar.activation(lnr[:], sqnr64_sb[:], AF.Ln)
        invn_r = work.tile([1, 64], F32, tag="invn_r")
        nc.scalar.activation(invn_r[:], lnr[:], AF.Exp, scale=-0.5)

        sqnc_ps = psS.tile([64, 1], F32, tag="small")   # sqn of anchor rows
        nc.tensor.matmul(sqnc_ps[:], sq_r[:], ones128, start=True, stop=True)
        sqnc_sb = const.tile([64, 1], F32, tag="sqnc_sb")
        nc.vector.tensor_copy(sqnc_sb[:], sqnc_ps[:])

        invnb_ps = psB.tile([128, 512], F32, tag="big")
        nc.tensor.matmul(invnb_ps[:], ones1, invn[:], start=True, stop=True)
        fT = const.tile([128, 512], F32, tag="fT")
        nc.vector.tensor_tensor(fT[:], cfT, invnb_ps[:], op=OP.mult)
        invnrb_ps = psB.tile([128, 64], F32, tag="big")
        nc.tensor.matmul(invnrb_ps[:], ones1, invn_r[:], start=True, stop=True)
        fTr = const.tile([128, 64], F32, tag="fTr")
        nc.vector.tensor_tensor(fTr[:], cfTr, invnrb_ps[:], op=OP.mult)

        # p broadcast [128, 512] (p_j along free dim on every partition)
        pb_ps = psB.tile([128, 512], F32, tag="big")
        nc.tensor.matmul(pb_ps[:], ones1, p_row, start=True, stop=True)
        p_b = const.tile([128, 512], F32, tag="p_b")
        nc.scalar.copy(p_b[:], pb_ps[:])

        negprowb = const.tile([128, 64], F32, tag="negprowb")
        nc.vector.tensor_scalar(negprowb[:], prowb, -1.0, None, op0=OP.mult)
        negp_col = const.tile([128, 4], F32, tag="negp_col")
        nc.vector.tensor_scalar(negp_col[:], p_col, -1.0, None, op0=OP.mult)
        negprow = const.tile([64, 1], F32, tag="negprow")
        nc.vector.tensor_scalar(negprow[:], prow, -1.0, None, op0=OP.mult)

        # ---------------- exp-similarity columns (transposed) ----------------
        eT = []
        for q in range(4):
            gT_ps = psB.tile([128, 64], F32, tag="big")
            nc.tensor.matmul(gT_ps[:], fT[:, 128 * q:128 * (q + 1)], fTr[:],
                             start=True, stop=True)
            e_q = const.tile([128, 64], F32, tag=f"eT{q}")
            nc.scalar.activation(e_q[:], gT_ps[:], AF.Exp, scale=1.0 / TEMP)
            nc.vector.tensor_tensor(e_q[:], e_q[:],
                                    tdiag[:, 64 * q:64 * (q + 1)], op=OP.mult)
            eT.append(e_q)

        # ---------------- pd structures ----------------
        pdT = []
        for q in range(4):
            t = const.tile([128, 64], F32, tag=f"pdT{q}")
            nc.scalar.activation(t[:], prowb, AF.Abs,
                                 bias=negp_col[:, q:q + 1])
            pdT.append(t)

        pd_rows = const.tile([64, 512], F32, tag="pd_rows")
        nc.scalar.activation(pd_rows[:], p_b[0:64, :], AF.Abs,
                             bias=negprow[:])

        th = work.tile([64, 512], F32, tag="th")
        nc.scalar.activation(th[:], pd_rows[:], AF.Tanh, scale=0.5)
        sw = work.tile([64, 512], F32, tag="sw")
        nc.vector.tensor_scalar(sw[:], th[:], 0.5, 0.5, op0=OP.mult, op1=OP.add)
        soft_wm = const.tile([64, 512], F32, tag="soft_wm")
        nc.vector.tensor_tensor(soft_wm[:], sw[:], vdiag, op=OP.mult)

        pmask = const.tile([64, 512], F32, tag="pmask")
        cnt_rows = const.tile([64, 1], F32, tag="cnt_rows")
        nc.vector.tensor_scalar(pmask[:], p_b[0:64, :], prow, None,
                                op0=OP.is_gt, op1=OP.add, accum_out=cnt_rows[:])

        # ---------------- OAL ----------------
        projc_ps = psS.tile([64, 1], F32, tag="small")
        nc.tensor.matmul(projc_ps[:], cfTr, v, start=True, stop=True)
        projc_sb = const.tile([64, 1], F32, tag="projc_sb")
        nc.vector.tensor_copy(projc_sb[:], projc_ps[:])

        projb_ps = psB.tile([64, 512], F32, tag="big")
        nc.tensor.matmul(projb_ps[:], vrep, cfT, start=True, stop=True)
        t3a = work.tile([64, 512], F32, tag="t3a")
        nc.vector.scalar_tensor_tensor(t3a[:], projb_ps[:], projc_sb[:],
                                       pmask[:], op0=OP.subtract, op1=OP.mult)

        rgram_ps = psB.tile([64, 512], F32, tag="big")
        nc.tensor.matmul(rgram_ps[:], cfTr, cfT, start=True, stop=True)
        t1 = work.tile([64, 512], F32, tag="t1")
        nc.vector.scalar_tensor_tensor(t1[:], rgram_ps[:], -2.0, sqnb_sb[:],
                                       op0=OP.mult, op1=OP.add)
        sqd0 = work.tile([64, 512], F32, tag="sqd0")
        nc.vector.tensor_scalar(sqd0[:], t1[:], sqnc_sb[:], None, op0=OP.add)
        sqd = work.tile([64, 512], F32, tag="sqd")
        nc.vector.tensor_scalar(sqd[:], sqd0[:], 1e-24, None, op0=OP.max)
        lnd = work.tile([64, 512], F32, tag="lnd")
        nc.scalar.activation(lnd[:], sqd[:], AF.Ln)
        invd = work.tile([64, 512], F32, tag="invd")
        nc.scalar.activation(invd[:], lnd[:], AF.Exp, scale=-0.5)

        t4 = work.tile([64, 512], F32, tag="t4")
        oal_rows = const.tile([64, 1], F32, tag="oal_rows")
        nc.vector.scalar_tensor_tensor(t4[:], t3a[:], 1.0, invd[:],
                                       op0=OP.mult, op1=OP.mult,
                                       accum_out=oal_rows[:])

        oal_ps = psS.tile([1, 1], F32, tag="small")
        nc.tensor.matmul(oal_ps[:], oal_rows[:], ones64, start=True, stop=True)
        nc.scalar.copy(out_tile[0:1, 1:2], oal_ps[:])
        cnt_ps = psS.tile([1, 1], F32, tag="small")
        nc.tensor.matmul(cnt_ps[:], cnt_rows[:], ones64, start=True, stop=True)
        nc.scalar.copy(out_tile[0:1, 2:3], cnt_ps[:])

        # gram of normalized rows, evacuated to SBUF before the pair loop
        gram_ps = psB.tile([64, 512], F32, tag="big")
        nc.tensor.matmul(gram_ps[:], fTr[:], fT[:], start=True, stop=True)
        gram_sb = const.tile([64, 512], F32, tag="gram_sb")
        nc.vector.tensor_copy(gram_sb[:], gram_ps[:])

        # ---------------- denominator pair loop ----------------
        obs_ps = psO.tile([1, 1], F32, tag="obs")
        dveobs = const.tile([1, 2], F32, tag="dveobs")
        ld_all = const.tile([2, HALF, 512], F32, tag=f"ld_all{rep % 2}")
        for r in range(HALF):
            pd_b = pdbp.tile([128, 512], F32, tag="pd_b")
            nc.scalar.activation(pd_b[:], p_b[:], AF.Abs,
                                 bias=negprowb[:, r:r + 1])
            # DVE observer: absorbs the ACT(pd_b) wait so the mask ops
            # below need only their PE slot-release wait.
            nc.vector.tensor_copy(dveobs[:], pd_b[0:1, 0:2])
            if r >= PBUF:
                # PE observer: waits on the Ln that frees this pair's PSUM
                # slot, so the q==0 matmul below needs only its DVE wait.
                nc.tensor.matmul(obs_ps[:], ld_all[0:2, r - PBUF, 0:1],
                                 ld_all[0:2, r - PBUF, 0:1],
                                 start=True, stop=True)
            pr_ps = psP.tile([2, 512], F32, tag="pair")
            for q in range(4):
                mk = maskp.tile([128, 512], F32, tag="mk")
                nc.vector.tensor_scalar(mk[:], pd_b[:], pdT[q][:, r:r + 1],
                                        None, op0=OP.is_le)
                nc.tensor.matmul(pr_ps[:], eT[q][:, r:r + HALF + 1:HALF], mk[:],
                                 start=(q == 0), stop=(q == 3))
            nc.scalar.activation(ld_all[0:2, r, :], pr_ps[:],
                                 AF.Ln, bias=eps_sb[:])
        ldenom = const.tile([64, 512], F32, tag=f"ldenom{rep % 2}")
        nc.sync.dma_start(ldenom[:], ld_all[:])
        # ACT observer of the assembly DMA so next-rep ACT writes to ld_all
        # need no extra DMA wait
        ldobs = const.tile([1, 2], F32, tag="ldobs")
        nc.scalar.copy(ldobs[:], ldenom[0:1, 0:2])

        # ---------------- RAL finalize ----------------
        t5 = work.tile([64, 512], F32, tag="t5")
        nc.vector.scalar_tensor_tensor(t5[:], gram_sb[:], -1.0 / TEMP, ldenom[:],
                                       op0=OP.mult, op1=OP.add)
        contrib = work.tile([64, 512], F32, tag="contrib")
        ral_rows = const.tile([64, 1], F32, tag="ral_rows")
        nc.vector.scalar_tensor_tensor(contrib[:], t5[:], 1.0, soft_wm[:],
                                       op0=OP.mult, op1=OP.mult,
                                       accum_out=ral_rows[:])
        ral_ps = psB.tile([1, 1], F32, tag="big")
        nc.tensor.matmul(ral_ps[:], ral_rows[:], ones64, start=True, stop=True)
        nc.scalar.copy(out_tile[0:1, 0:1], ral_ps[:])

        if last:
            nc.sync.dma_start(out_d[0:1, 0:4], out_tile[0:1, 0:4])


def _split_multiwaits(nc):
    """TPB instructions encode a single semaphore wait, but Tile emits up
    to one wait per dependency. Two legalizations: (1) drop same-engine
    self-waits that are provably satisfied (the engine completes its own
    instructions in order, so a wait on its own semaphore for a value
    already accumulated upstream is vacuous; DMA-queue semaphores are
    excluded since their increments fire on async transfer completion);
    (2) the kernel-tail drain keeps only the final output-DMA completion
    wait — every other wait is transitively implied by the PE -> ACT ->
    out-DMA chain."""
    eng_sem = {
        "EngineType.DVE": "DVE",
        "EngineType.Activation": "Activation",
        "EngineType.PE": "PE",
        "EngineType.Pool": "Pool",
        "EngineType.SP": "SP",
    }
    fn = nc.m.functions[0]
    streams = {}
    for blk in fn.blocks:
        for ins in blk.instructions:
            streams.setdefault(str(getattr(ins, "engine", None)), []).append(ins)
    for eng, insts in streams.items():
        own = eng_sem.get(eng)
        cum = {}
        last_dma_updates = set()
        for ins in insts:
            si = ins.sync_info
            if si is None:
                continue
            if type(ins).__name__ == "InstDMACopy":
                last_dma_updates = {u.id for u in si.on_update}
                own_q = {u.id for u in si.on_update}
                if len(si.on_wait) > 2:
                    # same-queue waits on earlier transfers are vacuous
                    # (HWDGE queues execute and complete in order)
                    new = [w for w in si.on_wait
                           if not (w.id in own_q
                                   and w.wait_value <= cum.get(w.id, 0))]
                    if len(new) > 1:
                        # ldenom assembly DMA across reps: its DVE wait
                        # (t5 of rep N-2 reading ldenom) and prior-rep DMA
                        # WAW wait are both transitively implied by its ACT
                        # wait: ld_all writes of rep N come after rep N-2's
                        # ral copy -> PE ral matmul -> contrib -> t5, and
                        # t5 itself waited on rep N-2's assembly DMA.
                        names = [w.ant_name for w in new]
                        assert any(n.startswith("Activation") for n in names), names
                        new = [w for w in new
                               if w.ant_name.startswith("Activation")]
                    assert len(new) <= 1, (
                        f"{ins.name} DMA still needs "
                        f"{[w.ant_name for w in new]}")
                    si.on_wait = new
                    ins.sync_info = si
                for u in si.on_update:
                    cum[u.id] = cum.get(u.id, 0) + u.update_value
                continue
            if len(si.on_wait) > 1:
                if type(ins).__name__ == "InstDrain":
                    kept = [w for w in si.on_wait if w.id in last_dma_updates]
                    assert kept, f"no DMA-completion wait for {ins.name}"
                    si.on_wait = kept[-1:]
                    ins.sync_info = si
                else:
                    new = [
                        w for w in si.on_wait
                        if not (own and w.ant_name.startswith(own + "_")
                                and w.wait_value <= cum.get(w.id, 0))
                    ]
                    assert len(new) <= 1, (
                        f"{ins.name} on {eng} still needs "
                        f"{[w.ant_name for w in new]}"
                    )
                    si.on_wait = new
                    ins.sync_info = si
            for u in si.on_update:
                cum[u.id] = cum.get(u.id, 0) + u.update_value


def _host_inputs(features, labels, v_prog):
    f32 = np.float32
    cf = np.ascontiguousarray(
        features.astype(f32).transpose(1, 0, 2).reshape(N, D))
    p = np.concatenate([labels, labels]).astype(f32)
    cfT = np.ascontiguousarray(cf.T)
    v = v_prog.astype(f32).reshape(128, 1)

    base = np.zeros((128, BLOB), f32)
    base[:, C_CFT[0]:C_CFT[1]] = cfT
    base[:, C_PCOL[0]:C_PCOL[1]] = p.reshape(4, 128).T
    base[:, C_V[0]:C_V[1]] = v
    base[:, C_VREP[0]:C_VREP[1]] = v
    base[:, C_ONES[0]:C_ONES[1]] = 1.0
    base[:, C_EPS[0]:C_EPS[1]] = EPS
    base[0, C_PROW512[0]:C_PROW512[1]] = p

    in_maps = []
    for c in range(NCORES):
        rows = np.r_[32 * c:32 * c + 32, 256 + 32 * c:256 + 32 * c + 32]
        prow = p[rows]
        blob = base.copy()
        blob[:, C_CFTR[0]:C_CFTR[1]] = cf[rows].T
        blob[:, C_PROWB[0]:C_PROWB[1]] = prow[None, :]
        blob[:64, C_PROWC[0]] = prow
        tdiag = np.ones((128, 256), f32)
        vdiag = np.ones((64, N), f32)
        for r, g in enumerate(rows):
            q, kp = divmod(g, 128)
            tdiag[kp, 64 * q + r] = 0.0
            vdiag[r, g] = 0.0
        blob[:, C_TDIAG[0]:C_TDIAG[1]] = tdiag
        blob[:64, C_VDIAG[0]:C_VDIAG[1]] = vdiag
        in_maps.append({"blob": blob})
    return in_maps


def _get_program(n_reps=1):
    key = ("nc", n_reps)
    if key not in _CACHE:
        _CACHE[key] = _build_program(n_reps)
    return _CACHE[key]


def _get_runner(n_reps=1):
    """Build (once) and cache a reusable jitted executable for the program.

    run_bass_kernel_spmd constructs a fresh jax.jit on every call, so each
    call pays retrace + compile-cache lookup + executable load.  Caching the
    jitted callable makes warm calls pure dispatch + device execution.
    """
    key = ("fn", n_reps)
    if key in _CACHE:
        return _CACHE[key]
    import jax
    from jax.sharding import Mesh, PartitionSpec
    from jax.experimental.shard_map import shard_map
    from concourse.bass2jax import (_bass_exec_p, partition_id_tensor,
                                    install_neuronx_cc_hook)

    install_neuronx_cc_hook()
    nc = _get_program(n_reps)
    partition_name = (nc.partition_id_tensor.name
                      if nc.partition_id_tensor else None)
    in_names, out_names, out_avals = [], [], []
    for alloc in nc.m.functions[0].allocations:
        if not isinstance(alloc, mybir.MemoryLocationSet):
            continue
        name = alloc.memorylocations[0].name
        if alloc.kind == "ExternalInput":
            if name != partition_name:
                in_names.append(name)
        elif alloc.kind == "ExternalOutput":
            shape = tuple(alloc.tensor_shape)
            dtype = mybir.dt.np(alloc.dtype)
            out_avals.append(jax.core.ShapedArray(shape, dtype))
            out_names.append(name)
    n_params = len(in_names)
    n_outs = len(out_avals)
    all_names = in_names + out_names
    if partition_name is not None:
        all_names.append(partition_name)
    donate = tuple(range(n_params, n_params + n_outs))

    def _body(*args):
        operands = list(args)
        if partition_name is not None:
            operands.append(partition_id_tensor())
        outs = _bass_exec_p.bind(
            *operands, out_avals=tuple(out_avals), in_names=tuple(all_names),
            out_names=tuple(out_names), lowering_input_output_aliases=(),
            sim_require_finite=True, sim_require_nnan=True, nc=nc)
        return tuple(outs)

    devices = jax.devices()[:NCORES]
    mesh = Mesh(np.asarray(devices), ("core",))
    in_specs = (PartitionSpec("core"),) * (n_params + n_outs)
    out_specs = (PartitionSpec("core"),) * len(out_names)
    fn = jax.jit(shard_map(_body, mesh=mesh, in_specs=in_specs,
                           out_specs=out_specs, check_rep=False),
                 donate_argnums=donate, keep_unused=True)
    runner = (fn, in_names, out_names, out_avals)
    _CACHE[key] = runner
    return runner


class _Res:
    def __init__(self, results):
        self.results = results


def kernel(features, labels, v_prog, _bench=None, _n_reps=1):
    in_maps = _host_inputs(np.asarray(features), np.asarray(labels),
                           np.asarray(v_prog))
    if _bench:
        nc = _get_program(_n_reps)
        res = run_bass_kernel_spmd(nc, in_maps, list(range(NCORES)),
                                   **dict(_bench))
    else:
        fn, in_names, out_names, out_avals = _get_runner(_n_reps)
        concat_in = [
            np.concatenate([np.asarray(in_maps[c][nm])
                            for c in range(NCORES)], axis=0)
            for nm in in_names
        ]
        concat_zeros = [
            np.zeros((NCORES * av.shape[0], *av.shape[1:]), av.dtype)
            for av in out_avals
        ]
        out_arrs = fn(*concat_in, *concat_zeros)
        res = _Res([
            {name: np.asarray(out_arrs[i]).reshape(
                NCORES, *out_avals[i].shape)[c]
             for i, name in enumerate(out_names)}
            for c in range(NCORES)
        ])
    parts = np.stack([res.results[c]["partials"][0] for c in range(NCORES)])
    f32 = np.float32
    ral = parts[:, 0].sum(dtype=f32) / f32(N * (N - 1))
    oal_num = (parts[:, 1] * parts[:, 3]).sum(dtype=f32)
    oal = -oal_num / parts[:, 2].sum(dtype=f32)
    out = np.float32(ral + oal)
    if _bench is not None:
        return out, res
    return out

